# revision 11
# baseline (speedup 1.0000x reference)
"""FFM (field-aware factorization machine) forward pass on 8 Trainium2 cores.

Math (per sample b):
    linear[b] = X[b,:] @ w1 + b0
    C[i,j]    = sum_k v[i, field[j], k] * v[j, field[i], k]   (pair coefficients)
    inter[b]  = sum_{i<j} C[i,j] X[b,i] X[b,j]
    out[b]    = sigmoid(linear[b] + inter[b])

Strategy:
  * Precompute Cm = strict-upper(C) on host (512x512, tiny vs the batched work).
  * inter[b] = rowsum((X @ Cm) * X).  The heavy op is the [B,512]x[512,512]
    matmul, data-parallel over batch across 8 cores (4096 rows/core).
  * Linear-term folding: row 511 of Cm is structurally zero (strict upper
    triangle), so store w1^T there and feed the matmul's stationary operand a
    1.0 in the matching X^T row.  Then Y = X@Cm + 1*w1^T and
    rowsum(Y * X) = inter + linear, with zero extra instructions.
  * Matmul operands in bf16 (full-rate PE, 4 accumulating K-passes into one
    fp32 PSUM bank per 128-row batch tile).
  * Fused epilogue: ONE VectorE scalar_tensor_tensor per tile computes
    (Y + 0) * X with accum_out = row-sum -> the whole multiply+reduce in a
    single 512-cycle instruction.  ScalarE only does batched sigmoids.
  * Whole X shard resident in SBUF.  DMA descriptor issue costs ~2.9us/MB on
    the issuing sequencer and one HWDGE queue sustains only ~150GB/s, so the
    load is split: sync queue carries C[k0,k1]+X^T, scalar queue carries
    C[k2,k3]+bias+X, ordered by first-use time.
  * 8 dummy matmuls at stream start warm the PE HAM clock gate (3.4us busy
    window) so real matmuls run at 2.4GHz from the first tile.

Raw bass (no TileContext: this container's walrus rejects Tile's multi-wait
encodings and the TENSOR_TENSOR_REDUCE direct-ISA opcode).
"""

import contextlib

import numpy as np
import ml_dtypes

P = 128          # partitions / tile rows
F = 512          # features
KT = F // P      # 4 contraction tiles
NCORES = 8
B = 32768
BSH = B // NCORES   # 4096 rows per core
NT = BSH // P       # 32 batch tiles per core
NPSUM = 4           # psum rotation depth
NWARM = 11          # dummy warm-up matmuls (~3.4us -> HAM K=8/8)

BF16 = ml_dtypes.bfloat16


def _groups(singles, pairs_until, quad):
    """DMA groups as (t0, ntiles): singles, then pairs, then bigger blocks.
    Granularity tracks first-use time: early tiles need low latency, late
    tiles want fewer descriptors."""
    gs = [(t, 1) for t in range(singles)]
    t = singles
    while t < pairs_until:
        gs.append((t, 2))
        t += 2
    while t < NT:
        n = min(quad, NT - t)
        gs.append((t, n))
        t += n
    return gs


XT_G = _groups(4, 8, 4)
XN_G = _groups(2, 8, 4)


def _build_bass():
    import concourse.bass as bass
    from concourse import mybir

    nc = bass.Bass()

    xnat = nc.declare_dram_parameter("xnat", [BSH, F], mybir.dt.bfloat16, isOutput=False)[:]
    xt = nc.declare_dram_parameter("xt", [NT, P, KT, P], mybir.dt.bfloat16, isOutput=False)[:]
    cmat = nc.declare_dram_parameter("cmat", [P, KT, F], mybir.dt.bfloat16, isOutput=False)[:]
    bias = nc.declare_dram_parameter("bias", [1], mybir.dt.float32, isOutput=False)[:]
    y = nc.declare_dram_parameter("y", [P, NT], mybir.dt.float32, isOutput=True)[:]

    # group views with tile index in the free dimension
    xn_pt = xnat.rearrange("(t p) f -> p t f", p=P)    # [P, NT, F]
    xt_pt = xt.rearrange("t p k b -> p t k b")         # [P, NT, KT, P]

    xt_of = {}
    for gi, (t0, n) in enumerate(XT_G):
        for t in range(t0, t0 + n):
            xt_of[t] = gi
    xn_of = {}
    for gi, (t0, n) in enumerate(XN_G):
        for t in range(t0, t0 + n):
            xn_of[t] = gi

    with contextlib.ExitStack() as st:
        ec = st.enter_context
        c_sb = ec(nc.sbuf_tensor([P, KT, F], mybir.dt.bfloat16))
        b_sb = ec(nc.sbuf_tensor([P, 1], mybir.dt.float32))
        xbuf = ec(nc.sbuf_tensor([P, NT, F], mybir.dt.bfloat16))
        xtbuf = ec(nc.sbuf_tensor([P, NT, KT, P], mybir.dt.bfloat16))
        dump = ec(nc.sbuf_tensor([P, F], mybir.dt.float32))
        acc = ec(nc.sbuf_tensor([P, NT], mybir.dt.float32))
        out_sb = ec(nc.sbuf_tensor([P, NT], mybir.dt.float32))
        ps = [ec(nc.psum_tensor(f"ps{i}", [P, F], mybir.dt.float32)) for i in range(NPSUM)]
        ps_warm = ec(nc.psum_tensor("ps_warm", [P, F], mybir.dt.float32))

        s_c = [ec(nc.semaphore(name=f"s_c{k}")) for k in range(KT)]
        s_b = ec(nc.semaphore(name="s_b"))
        s_xt = [ec(nc.semaphore(name=f"s_xt{i}")) for i in range(len(XT_G))]
        s_xn = [ec(nc.semaphore(name=f"s_xn{i}")) for i in range(len(XN_G))]
        s_mm = ec(nc.semaphore(name="s_mm"))
        s_mul = ec(nc.semaphore(name="s_mul"))
        s_act = ec(nc.semaphore(name="s_act"))
        s_out = ec(nc.semaphore(name="s_out"))

        block = ec(nc.Block())

        def xt_issue(eng, gi):
            t0, n = XT_G[gi]
            eng.dma_start(
                out=xtbuf[:, t0 : t0 + n, :, :], in_=xt_pt[:, t0 : t0 + n, :, :]
            ).then_inc(s_xt[gi], 16)

        def xn_issue(eng, gi):
            t0n, nn = XN_G[gi]
            eng.dma_start(
                out=xbuf[:, t0n : t0n + nn, :], in_=xn_pt[:, t0n : t0n + nn, :]
            ).then_inc(s_xn[gi], 16)

        n_xt_tail = 2   # xt groups moved to the scalar queue
        n_xn_tail = 2   # xn groups moved to the gpsimd (SWDGE) queue

        @block.sync
        def _(sync):
            # sync HWDGE queue: C k0,k1 then X^T groups (PE's diet)
            for k in (0, 1):
                sync.dma_start(out=c_sb[:, k, :], in_=cmat[:, k, :]).then_inc(s_c[k], 16)
            for gi in range(len(XT_G) - n_xt_tail):
                xt_issue(sync, gi)
            sync.wait_ge(s_act, NT // 4)
            sync.dma_start(out=y, in_=out_sb[:]).then_inc(s_out, 16)
            sync.wait_ge(s_out, 16)

        @block.gpsimd
        def _(gpsimd):
            # gpsimd SWDGE queue: tail X groups (needed last)
            for gi in range(len(XN_G) - n_xn_tail, len(XN_G)):
                xn_issue(gpsimd, gi)

        @block.tensor
        def _(tensor):
            for _w in range(NWARM):
                nc.tensor.matmul(
                    ps_warm[:], xtbuf[:, 0, 0, :], c_sb[:, 0, :], start=True, stop=True
                )
            for k in range(KT):
                tensor.wait_ge(s_c[k], 16)
            for t in range(NT):
                gi = xt_of[t]
                if t == XT_G[gi][0]:
                    tensor.wait_ge(s_xt[gi], 16)
                if t >= NPSUM:
                    # psum bank reuse: epilogue(t-NPSUM) must be done reading
                    tensor.wait_ge(s_mul, t - NPSUM + 1)
                pst = ps[t % NPSUM]
                for k in range(KT):
                    mm = nc.tensor.matmul(
                        pst[:],
                        xtbuf[:, t, k, :],
                        c_sb[:, k, :],
                        start=(k == 0),
                        stop=(k == KT - 1),
                    )
                mm.then_inc(s_mm, 1)

        @block.vector
        def _(vector):
            for t in range(NT):
                vector.wait_ge(s_mm, t + 1)
                gi = xn_of[t]
                if t == XN_G[gi][0]:
                    vector.wait_ge(s_xn[gi], 16)
                # fused (Y + 0) * X with accum_out = rowsum -> acc[:, t]
                nc.vector.scalar_tensor_tensor(
                    out=dump[:],
                    in0=ps[t % NPSUM][:],
                    scalar=0.0,
                    in1=xbuf[:, t, :],
                    op0=mybir.AluOpType.add,
                    op1=mybir.AluOpType.mult,
                    accum_out=acc[:, t : t + 1],
                ).then_inc(s_mul, 1)

        @block.scalar
        def _(scalar):
            # scalar HWDGE queue: C k2,k3 + bias + X groups + tail X^T groups
            for k in (2, 3):
                scalar.dma_start(out=c_sb[:, k, :], in_=cmat[:, k, :]).then_inc(s_c[k], 16)
            scalar.dma_start(out=b_sb[:], in_=bias.to_broadcast([P, 1])).then_inc(s_b, 16)
            for gi in range(len(XN_G) - n_xn_tail):
                xn_issue(scalar, gi)
            for gi in range(len(XT_G) - n_xt_tail, len(XT_G)):
                xt_issue(scalar, gi)
            scalar.wait_ge(s_b, 16)
            for g in range(NT // 4):
                # all 4 accs of the batch ready (s_mul counts STTs in order)
                scalar.wait_ge(s_mul, 4 * g + 4)
                nc.scalar.activation(
                    out=out_sb[:, 4 * g : 4 * g + 4],
                    in_=acc[:, 4 * g : 4 * g + 4],
                    func=mybir.ActivationFunctionType.Sigmoid,
                    bias=b_sb[:],
                    scale=1.0,
                ).then_inc(s_act, 1)

    return nc


def _host_prep(X, w1, b, v, feature2field):
    """Returns per-core input maps."""
    X = np.asarray(X, dtype=np.float32)
    w1 = np.asarray(w1, dtype=np.float32)
    b = np.asarray(b, dtype=np.float32)
    v = np.asarray(v, dtype=np.float32)
    f2f = np.asarray(feature2field, dtype=np.int32)

    # Pair-coefficient matrix: C[i,j] = sum_k v[i, f2f[j], k] * v[j, f2f[i], k]
    A = v[:, f2f, :]                      # [n, n, k]
    C = (A * A.transpose(1, 0, 2)).sum(axis=2)
    Cm = np.triu(C, 1)
    # Fold the linear term: row F-1 of strict-upper Cm is all zeros.
    Cm[F - 1, :] = w1[:, 0]
    c_bf = Cm.astype(BF16)
    # SBUF layout [p, k, j] with row index i = k*P + p
    c_host = np.ascontiguousarray(c_bf.reshape(KT, P, F).transpose(1, 0, 2))

    X_bf = X.astype(BF16)
    in_maps = []
    for c in range(NCORES):
        Xc = X_bf[c * BSH : (c + 1) * BSH]            # [4096, 512]
        xnat = np.ascontiguousarray(Xc)
        # XT layout [t, p, k, b] = Xc[t*P + b, k*P + p]
        xtl = np.ascontiguousarray(Xc.reshape(NT, P, KT, P).transpose(0, 3, 2, 1))
        # stationary-operand row 511 (k=KT-1, p=P-1) := 1.0 for the w1 fold
        xtl[:, P - 1, KT - 1, :] = BF16(1.0)
        in_maps.append({"xnat": xnat, "xt": xtl, "cmat": c_host, "bias": b})
    return in_maps


def _run(in_maps, trace=False):
    from concourse.bass_utils import run_bass_kernel_spmd

    nc = _build_bass()
    res = run_bass_kernel_spmd(nc, in_maps, core_ids=list(range(NCORES)), trace=trace)
    out = np.concatenate([r["y"].reshape(P, NT).T.reshape(-1) for r in res.results])
    return out, res


def kernel(X, w1, b, v, feature2field):
    in_maps = _host_prep(X, w1, b, v, feature2field)
    out, _ = _run(in_maps, trace=False)
    return out.astype(np.float32)


if __name__ == "__main__":
    pass


# revision 12
# speedup vs baseline: 1.0295x; 1.0295x over previous
"""FFM (field-aware factorization machine) forward pass on 8 Trainium2 cores.

Math (per sample b):
    linear[b] = X[b,:] @ w1 + b0
    C[i,j]    = sum_k v[i, field[j], k] * v[j, field[i], k]   (pair coefficients)
    inter[b]  = sum_{i<j} C[i,j] X[b,i] X[b,j]
    out[b]    = sigmoid(linear[b] + inter[b])

Strategy:
  * Precompute Cm = strict-upper(C) on host (512x512, tiny vs the batched work).
  * inter[b] = rowsum((X @ Cm) * X).  The heavy op is the [B,512]x[512,512]
    matmul, data-parallel over batch across 8 cores (4096 rows/core).
  * Linear-term folding: row 511 of Cm is structurally zero (strict upper
    triangle), so store w1^T there and feed the matmul's stationary operand a
    1.0 in the matching X^T row.  Then Y = X@Cm + 1*w1^T and
    rowsum(Y * X) = inter + linear, with zero extra instructions.
  * Matmul operands in bf16 (full-rate PE, 4 accumulating K-passes into one
    fp32 PSUM bank per 128-row batch tile).
  * Fused epilogue: ONE VectorE scalar_tensor_tensor per tile computes
    (Y + 0) * X with accum_out = row-sum -> the whole multiply+reduce in a
    single 512-cycle instruction.  ScalarE only does batched sigmoids.
  * Whole X shard resident in SBUF.  DMA descriptor issue costs ~2.9us/MB on
    the issuing sequencer and one HWDGE queue sustains only ~150GB/s, so the
    load is split: sync queue carries C[k0,k1]+X^T, scalar queue carries
    C[k2,k3]+bias+X, ordered by first-use time.
  * 8 dummy matmuls at stream start warm the PE HAM clock gate (3.4us busy
    window) so real matmuls run at 2.4GHz from the first tile.

Raw bass (no TileContext: this container's walrus rejects Tile's multi-wait
encodings and the TENSOR_TENSOR_REDUCE direct-ISA opcode).
"""

import contextlib

import numpy as np
import ml_dtypes

P = 128          # partitions / tile rows
F = 512          # features
KT = F // P      # 4 contraction tiles
NCORES = 8
B = 32768
BSH = B // NCORES   # 4096 rows per core
NT = BSH // P       # 32 batch tiles per core
NPSUM = 8           # psum banks (all of PSUM; pair-windows, 4 in flight)
NWARM = 4           # dummy warm-up matmuls bridging the first DMA arrivals

BF16 = ml_dtypes.bfloat16


def _groups(singles, pairs_until, quad):
    """DMA groups as (t0, ntiles): singles, then pairs, then bigger blocks.
    Granularity tracks first-use time: early tiles need low latency, late
    tiles want fewer descriptors."""
    gs = [(t, 1) for t in range(singles)]
    t = singles
    while t < pairs_until:
        gs.append((t, 2))
        t += 2
    while t < NT:
        n = min(quad, NT - t)
        gs.append((t, n))
        t += n
    return gs


XT_G = _groups(4, 8, 4)
XN_G = _groups(2, 8, 4)


def _build_bass():
    import concourse.bass as bass
    from concourse import mybir

    nc = bass.Bass()

    xnat = nc.declare_dram_parameter("xnat", [BSH, F], mybir.dt.bfloat16, isOutput=False)[:]
    xt = nc.declare_dram_parameter("xt", [NT, P, KT, P], mybir.dt.bfloat16, isOutput=False)[:]
    cmat = nc.declare_dram_parameter("cmat", [P, KT, F], mybir.dt.bfloat16, isOutput=False)[:]
    bias = nc.declare_dram_parameter("bias", [1], mybir.dt.float32, isOutput=False)[:]
    y = nc.declare_dram_parameter("y", [P, NT], mybir.dt.float32, isOutput=True)[:]

    # group views with tile index in the free dimension
    xn_pt = xnat.rearrange("(t p) f -> p t f", p=P)    # [P, NT, F]
    xt_pt = xt.rearrange("t p k b -> p t k b")         # [P, NT, KT, P]

    xt_of = {}
    for gi, (t0, n) in enumerate(XT_G):
        for t in range(t0, t0 + n):
            xt_of[t] = gi
    xn_of = {}
    for gi, (t0, n) in enumerate(XN_G):
        for t in range(t0, t0 + n):
            xn_of[t] = gi

    with contextlib.ExitStack() as st:
        ec = st.enter_context
        c_sb = ec(nc.sbuf_tensor([P, KT, F], mybir.dt.bfloat16))
        b_sb = ec(nc.sbuf_tensor([P, 1], mybir.dt.float32))
        xbuf = ec(nc.sbuf_tensor([P, NT, F], mybir.dt.bfloat16))
        xtbuf = ec(nc.sbuf_tensor([P, NT, KT, P], mybir.dt.bfloat16))
        dump = ec(nc.sbuf_tensor([P, F], mybir.dt.float32))
        acc = ec(nc.sbuf_tensor([P, NT], mybir.dt.float32))
        out_sb = ec(nc.sbuf_tensor([P, NT], mybir.dt.float32))
        ps = [ec(nc.psum_tensor(f"ps{i}", [P, F], mybir.dt.float32)) for i in range(NPSUM)]
        ps_warm = ps[NPSUM - 1]  # warmups run before any real use of the last bank

        s_c = [ec(nc.semaphore(name=f"s_c{k}")) for k in range(KT)]
        s_b = ec(nc.semaphore(name="s_b"))
        s_xt = [ec(nc.semaphore(name=f"s_xt{i}")) for i in range(len(XT_G))]
        s_xn = [ec(nc.semaphore(name=f"s_xn{i}")) for i in range(len(XN_G))]
        s_mm = ec(nc.semaphore(name="s_mm"))
        s_mul = ec(nc.semaphore(name="s_mul"))
        s_act = ec(nc.semaphore(name="s_act"))
        s_out = ec(nc.semaphore(name="s_out"))

        block = ec(nc.Block())

        def xt_issue(eng, gi):
            t0, n = XT_G[gi]
            eng.dma_start(
                out=xtbuf[:, t0 : t0 + n, :, :], in_=xt_pt[:, t0 : t0 + n, :, :]
            ).then_inc(s_xt[gi], 16)

        def xn_issue(eng, gi):
            t0n, nn = XN_G[gi]
            eng.dma_start(
                out=xbuf[:, t0n : t0n + nn, :], in_=xn_pt[:, t0n : t0n + nn, :]
            ).then_inc(s_xn[gi], 16)

        n_xt_tail = 2   # xt groups moved to the scalar queue
        n_xn_tail = 2   # xn groups moved to the gpsimd (SWDGE) queue

        @block.sync
        def _(sync):
            # sync HWDGE queue: C k0,k1 then X^T groups (PE's diet)
            for k in (0, 1):
                sync.dma_start(out=c_sb[:, k, :], in_=cmat[:, k, :]).then_inc(s_c[k], 16)
            for gi in range(len(XT_G) - n_xt_tail):
                xt_issue(sync, gi)
            sync.wait_ge(s_act, NT // 4)
            sync.dma_start(out=y, in_=out_sb[:]).then_inc(s_out, 16)
            sync.wait_ge(s_out, 16)

        @block.gpsimd
        def _(gpsimd):
            # gpsimd SWDGE queue: tail X groups.  Delay issue until the PE is
            # several tiles in so these transfers don't steal HBM bandwidth
            # from the latency-critical C + first-tile loads.
            gpsimd.wait_ge(s_mm, 6)
            for gi in range(len(XN_G) - n_xn_tail, len(XN_G)):
                xn_issue(gpsimd, gi)

        @block.tensor
        def _(tensor):
            for _w in range(NWARM):
                nc.tensor.matmul(
                    ps_warm[:], xtbuf[:, 0, 0, :], c_sb[:, 0, :], start=True, stop=True
                )
            # pair-windows, k-outer: window w = tiles (2w, 2w+1) in banks
            # (2(w%4), 2(w%4)+1).  k is the outer loop inside a window so the
            # first real matmuls need only C[k0] + xt tile0 -- the remaining
            # C chunks stream in while k0/k1 passes run.
            NW = NT // 2
            for w in range(NW):
                t0, t1 = 2 * w, 2 * w + 1
                b0 = 2 * (w % 4)
                for t in (t0, t1):
                    gi = xt_of[t]
                    if t == XT_G[gi][0]:
                        tensor.wait_ge(s_xt[gi], 16)
                if w >= 4:
                    # bank reuse: epilogues of window w-4 must be done
                    tensor.wait_ge(s_mul, 2 * (w - 4) + 2)
                for k in range(KT):
                    if w < 3:
                        tensor.wait_ge(s_c[k], 16)
                    for i, t in enumerate((t0, t1)):
                        mm = nc.tensor.matmul(
                            ps[b0 + i][:],
                            xtbuf[:, t, k, :],
                            c_sb[:, k, :],
                            start=(k == 0),
                            stop=(k == KT - 1),
                            skip_group_check=True,
                        )
                        if k == KT - 1:
                            mm.then_inc(s_mm, 1)

        @block.vector
        def _(vector):
            for t in range(NT):
                vector.wait_ge(s_mm, t + 1)
                gi = xn_of[t]
                if t == XN_G[gi][0]:
                    vector.wait_ge(s_xn[gi], 16)
                # fused (Y + 0) * X with accum_out = rowsum -> acc[:, t]
                nc.vector.scalar_tensor_tensor(
                    out=dump[:],
                    in0=ps[2 * ((t // 2) % 4) + (t % 2)][:],
                    scalar=0.0,
                    in1=xbuf[:, t, :],
                    op0=mybir.AluOpType.add,
                    op1=mybir.AluOpType.mult,
                    accum_out=acc[:, t : t + 1],
                ).then_inc(s_mul, 1)

        @block.scalar
        def _(scalar):
            # scalar HWDGE queue: C k2,k3 + bias + X groups + tail X^T groups
            for k in (2, 3):
                scalar.dma_start(out=c_sb[:, k, :], in_=cmat[:, k, :]).then_inc(s_c[k], 16)
            scalar.dma_start(out=b_sb[:], in_=bias.to_broadcast([P, 1])).then_inc(s_b, 16)
            for gi in range(len(XN_G) - n_xn_tail):
                xn_issue(scalar, gi)
            for gi in range(len(XT_G) - n_xt_tail, len(XT_G)):
                xt_issue(scalar, gi)
            scalar.wait_ge(s_b, 16)
            for g in range(NT // 4):
                # all 4 accs of the batch ready (s_mul counts STTs in order)
                scalar.wait_ge(s_mul, 4 * g + 4)
                nc.scalar.activation(
                    out=out_sb[:, 4 * g : 4 * g + 4],
                    in_=acc[:, 4 * g : 4 * g + 4],
                    func=mybir.ActivationFunctionType.Sigmoid,
                    bias=b_sb[:],
                    scale=1.0,
                ).then_inc(s_act, 1)

    return nc


def _host_prep(X, w1, b, v, feature2field):
    """Returns per-core input maps."""
    X = np.asarray(X, dtype=np.float32)
    w1 = np.asarray(w1, dtype=np.float32)
    b = np.asarray(b, dtype=np.float32)
    v = np.asarray(v, dtype=np.float32)
    f2f = np.asarray(feature2field, dtype=np.int32)

    # Pair-coefficient matrix: C[i,j] = sum_k v[i, f2f[j], k] * v[j, f2f[i], k]
    A = v[:, f2f, :]                      # [n, n, k]
    C = (A * A.transpose(1, 0, 2)).sum(axis=2)
    Cm = np.triu(C, 1)
    # Fold the linear term: row F-1 of strict-upper Cm is all zeros.
    Cm[F - 1, :] = w1[:, 0]
    c_bf = Cm.astype(BF16)
    # SBUF layout [p, k, j] with row index i = k*P + p
    c_host = np.ascontiguousarray(c_bf.reshape(KT, P, F).transpose(1, 0, 2))

    X_bf = X.astype(BF16)
    in_maps = []
    for c in range(NCORES):
        Xc = X_bf[c * BSH : (c + 1) * BSH]            # [4096, 512]
        xnat = np.ascontiguousarray(Xc)
        # XT layout [t, p, k, b] = Xc[t*P + b, k*P + p]
        xtl = np.ascontiguousarray(Xc.reshape(NT, P, KT, P).transpose(0, 3, 2, 1))
        # stationary-operand row 511 (k=KT-1, p=P-1) := 1.0 for the w1 fold
        xtl[:, P - 1, KT - 1, :] = BF16(1.0)
        in_maps.append({"xnat": xnat, "xt": xtl, "cmat": c_host, "bias": b})
    return in_maps


def _run(in_maps, trace=False):
    from concourse.bass_utils import run_bass_kernel_spmd

    nc = _build_bass()
    res = run_bass_kernel_spmd(nc, in_maps, core_ids=list(range(NCORES)), trace=trace)
    out = np.concatenate([r["y"].reshape(P, NT).T.reshape(-1) for r in res.results])
    return out, res


def kernel(X, w1, b, v, feature2field):
    in_maps = _host_prep(X, w1, b, v, feature2field)
    out, _ = _run(in_maps, trace=False)
    return out.astype(np.float32)


if __name__ == "__main__":
    pass
